# revision 2
# baseline (speedup 1.0000x reference)
"""Causal attention head on 8 trn2 NeuronCores.

Sharding: core c = (batch b = c//2, type t = c%2). Each core handles 4
query stripes of 512 (one slot per stripe) of its batch against all keys
it needs. Causal balance: type A gets stripes with nkb {32,24,12,4},
type B {28,20,16,8} (72 key-blocks each). One SPMD program: every core
runs the padded template T = {32,24,16,8} (80 units); masking and
padding are driven purely by per-core input data (additive mask tiles),
so the instruction stream is identical on all cores.

Device computes, per slot j (query stripe q0..q0+512):
  sT[u]  = (k[kb=u] @ qT_stripe) /-  scores transposed [128 keys, 512 q]
  e      = exp(0.125*sT + mask)      bf16
  outT  += v[kb=u].T-halves @ e      [256 e, 512 q] psum accumulate
  r     += ones.T @ e                [1, 512] softmax denominators
Host finishes: out = (outT / r).T per stripe (pure data movement + tiny
elementwise; all matmuls/softmax on device).
"""

import sys

sys.path.insert(0, "/opt/trn_rl_repo")

import numpy as np
import ml_dtypes

B, S, DM, DQ = 4, 4096, 256, 64
T = [32, 24, 16, 8]  # padded template: key-blocks per slot
R_A = [32, 24, 12, 4]  # real key-blocks, type A cores
R_B = [28, 20, 16, 8]  # type B
NEG = np.float32(-1e9)

_CACHE = {}


def _q0s(R):
    return [(r - 4) * 128 for r in R]


def _build_nc():
    import concourse.bass as bass  # noqa: F401
    import concourse.tile as tile
    from concourse import bacc, mybir

    dt = mybir.dt
    f32, bf = dt.float32, dt.bfloat16
    f32r = dt.float32r

    nc = bacc.Bacc(
        "TRN2",
        target_bir_lowering=False,
        debug=False,
        enable_asserts=False,
        num_devices=8,
    )

    def din(name, shape, d=None):
        d = f32r if d is None else d
        return nc.dram_tensor(name, shape, d, kind="ExternalInput").ap()

    eq = din("eq", [256, 2048])
    ek = din("ek", [256, 4096])
    ev = din("ev", [256, 4096])
    wq = din("wq", [256, 64])
    wk = din("wk", [256, 64])
    wv = din("wv", [256, 256])
    mi = din("mi", [128, 4 * 8 * 512], bf)
    outT = nc.dram_tensor("outT", [256, 2048], f32, kind="ExternalOutput").ap()
    rs = nc.dram_tensor("rs", [1, 2048], f32, kind="ExternalOutput").ap()

    with tile.TileContext(nc) as tc:
        from contextlib import ExitStack

        with ExitStack() as ctx:
            const = ctx.enter_context(tc.tile_pool(name="const", bufs=1))

            # ---- persistent SBUF tensors ----
            eq_sb = [const.tile([128, 2048], f32r, tag=f"eq{h}", name=f"eq{h}") for h in range(2)]
            ek_sb = [const.tile([128, 4096], f32r, tag=f"ek{h}", name=f"ek{h}") for h in range(2)]
            ev_sb = [const.tile([128, 4096], f32r, tag=f"ev{h}", name=f"ev{h}") for h in range(2)]
            wq_sb = const.tile([128, 128], f32r, tag="wq", name="wq")  # [dm_half -> cols]
            wk_sb = const.tile([128, 128], f32r, tag="wk", name="wk")
            wv_sb = const.tile([128, 512], f32r, tag="wv", name="wv")
            m_sb = const.tile([128, 4 * 8 * 512], bf, tag="mi", name="mi")
            kT = const.tile([64, 4096], f32r, tag="kT", name="kT")
            qT = const.tile([64, 2048], f32r, tag="qT", name="qT")
            v_sb = const.tile([128, 32 * 256], bf, tag="v", name="v")
            ones_sb = const.tile([128, 1], bf, tag="ones", name="ones")

            for h in range(2):
                nc.sync.dma_start(
                    wq_sb[:, h * 64 : (h + 1) * 64], wq[h * 128 : (h + 1) * 128, :]
                )
                nc.sync.dma_start(
                    wk_sb[:, h * 64 : (h + 1) * 64], wk[h * 128 : (h + 1) * 128, :]
                )
                nc.sync.dma_start(
                    wv_sb[:, h * 256 : (h + 1) * 256], wv[h * 128 : (h + 1) * 128, :]
                )
            for h in range(2):
                for c in range(8):
                    cs = slice(c * 512, (c + 1) * 512)
                    nc.sync.dma_start(
                        ek_sb[h][:, cs], ek[h * 128 : (h + 1) * 128, cs]
                    )
            for h in range(2):
                for c in range(4):
                    cs = slice(c * 512, (c + 1) * 512)
                    nc.sync.dma_start(
                        eq_sb[h][:, cs], eq[h * 128 : (h + 1) * 128, cs]
                    )
            for h in range(2):
                for c in range(8):
                    cs = slice(c * 512, (c + 1) * 512)
                    nc.sync.dma_start(
                        ev_sb[h][:, cs], ev[h * 128 : (h + 1) * 128, cs]
                    )
            for c in range(8):
                cs = slice(c * 2048, (c + 1) * 2048)
                nc.sync.dma_start(m_sb[:, cs], mi[:, cs])
            nc.vector.memset(ones_sb[:], 1.0)

            def r32(ap):
                return ap  # tensors already f32r

            # ---- projections (K=256 contraction in two 128 passes) ----
            with tc.tile_pool(name="pp", bufs=3, space="PSUM") as pp:
                for c in range(8):  # kT = WkT.T @ ekT
                    ps = pp.tile([64, 512], f32, tag="ps", name="ps")
                    for h in range(2):
                        nc.tensor.matmul(
                            ps[:],
                            r32(wk_sb[:, h * 64 : (h + 1) * 64]),
                            r32(ek_sb[h][:, c * 512 : (c + 1) * 512]),
                            start=(h == 0),
                            stop=(h == 1),
                        )
                    nc.vector.tensor_copy(kT[:, c * 512 : (c + 1) * 512], ps[:])
                for c in range(4):  # qT
                    ps = pp.tile([64, 512], f32, tag="ps", name="ps")
                    for h in range(2):
                        nc.tensor.matmul(
                            ps[:],
                            r32(wq_sb[:, h * 64 : (h + 1) * 64]),
                            r32(eq_sb[h][:, c * 512 : (c + 1) * 512]),
                            start=(h == 0),
                            stop=(h == 1),
                        )
                    nc.vector.tensor_copy(qT[:, c * 512 : (c + 1) * 512], ps[:])
                for t in range(32):  # v natural [keys, 256] in bf16
                    ps = pp.tile([128, 256], f32, tag="ps", name="ps")
                    for h in range(2):
                        nc.tensor.matmul(
                            ps[:],
                            r32(ev_sb[h][:, t * 128 : (t + 1) * 128]),
                            r32(wv_sb[:, h * 256 : (h + 1) * 256]),
                            start=(h == 0),
                            stop=(h == 1),
                        )
                    nc.vector.tensor_copy(v_sb[:, t * 256 : (t + 1) * 256], ps[:])

            # ---- attention ----
            ps_pool = ctx.enter_context(
                tc.tile_pool(name="psc", bufs=2, space="PSUM")
            )
            po_pool = ctx.enter_context(
                tc.tile_pool(name="po", bufs=1, space="PSUM")
            )
            pr_pool = ctx.enter_context(
                tc.tile_pool(name="pr", bufs=1, space="PSUM")
            )
            epool = ctx.enter_context(tc.tile_pool(name="e", bufs=4))
            opool = ctx.enter_context(tc.tile_pool(name="o", bufs=2))

            Exp = mybir.ActivationFunctionType.Exp
            for j in range(4):
                tj = T[j]
                ndu = tj // 2
                po0 = po_pool.tile([128, 512], f32, tag="po0", name="po0")
                po1 = po_pool.tile([128, 512], f32, tag="po1", name="po1")
                pr = pr_pool.tile([1, 512], f32, tag="pr", name="pr")
                qs = r32(qT[:, j * 512 : (j + 1) * 512])
                es = [None] * ndu
                for d in range(ndu + 2):
                    if d < ndu:
                        ps = ps_pool.tile([128, 1024], f32, tag="ps", name="ps")
                        for half in range(2):
                            u = 2 * d + half
                            sl = ps[:, half * 512 : (half + 1) * 512]
                            nc.tensor.matmul(
                                sl,
                                r32(kT[:, u * 128 : (u + 1) * 128]),
                                qs,
                                start=True,
                                stop=True,
                            )
                        e = epool.tile([128, 1024], bf, tag="e", name="e")
                        nc.scalar.activation(e[:], ps[:], Exp, scale=0.125)
                        for half in range(2):
                            u = 2 * d + half
                            if u >= tj - 8:
                                p = u - (tj - 8)
                                col = (j * 8 + p) * 512
                                eh = e[:, half * 512 : (half + 1) * 512]
                                nc.vector.tensor_mul(
                                    eh, eh, m_sb[:, col : col + 512]
                                )
                        es[d] = e
                    if d > 1:
                        e = es[d - 2]
                        first, last = (d - 2 == 0), (d - 2 == ndu - 1)
                        for half in range(2):
                            u = 2 * (d - 2) + half
                            eh = e[:, half * 512 : (half + 1) * 512]
                            nc.tensor.matmul(
                                po0[:],
                                v_sb[:, u * 256 : u * 256 + 128],
                                eh,
                                start=(first and half == 0),
                                stop=(last and half == 1),
                            )
                            nc.tensor.matmul(
                                po1[:],
                                v_sb[:, u * 256 + 128 : (u + 1) * 256],
                                eh,
                                start=(first and half == 0),
                                stop=(last and half == 1),
                            )
                            nc.tensor.matmul(
                                pr[:],
                                ones_sb[:],
                                eh,
                                start=(first and half == 0),
                                stop=(last and half == 1),
                            )
                o0 = opool.tile([128, 512], f32, tag="o0", name="o0")
                o1 = opool.tile([128, 512], f32, tag="o1", name="o1")
                rr = opool.tile([1, 512], f32, tag="rr", name="rr")
                nc.vector.tensor_copy(o0[:], po0[:])
                nc.vector.tensor_copy(o1[:], po1[:])
                nc.vector.tensor_copy(rr[:], pr[:])
                nc.sync.dma_start(outT[0:128, j * 512 : (j + 1) * 512], o0[:])
                nc.sync.dma_start(outT[128:256, j * 512 : (j + 1) * 512], o1[:])
                nc.sync.dma_start(rs[:, j * 512 : (j + 1) * 512], rr[:])

    nc.compile()
    return nc


def _mask_for(Rj, Tj, q0):
    """bf16 [128, 8*512] additive mask for the last 8 units of a slot."""
    out = np.zeros((128, 8 * 512), dtype=np.float32)
    kp = np.arange(128)[:, None]
    qi = q0 + np.arange(512)[None, :]
    for p in range(8):
        u = Tj - 8 + p
        key = u * 128 + kp
        out[:, p * 512 : (p + 1) * 512] = np.where(
            key > qi, np.float32(0), np.float32(1)
        )
    return out


def kernel(encodings_for_q, encodings_for_k, encodings_for_v, mask, Wq, Wk, Wv):
    from concourse.bass_utils import run_bass_kernel_spmd

    if "nc" not in _CACHE:
        _CACHE["nc"] = _build_nc()
    nc = _CACHE["nc"]

    bf = ml_dtypes.bfloat16
    wq = np.ascontiguousarray(Wq.T, dtype=np.float32)
    wk = np.ascontiguousarray(Wk.T, dtype=np.float32)
    wv = np.ascontiguousarray(Wv.T, dtype=np.float32)

    in_maps = []
    metas = []
    for c in range(8):
        b, t = c // 2, c % 2
        R = R_A if t == 0 else R_B
        q0s = _q0s(R)
        eqT = np.concatenate(
            [encodings_for_q[b, q0 : q0 + 512, :].T for q0 in q0s], axis=1
        )
        mi = np.concatenate(
            [_mask_for(R[j], T[j], q0s[j]) for j in range(4)], axis=1
        ).astype(bf)
        in_maps.append(
            {
                "eq": np.ascontiguousarray(eqT, dtype=np.float32),
                "ek": np.ascontiguousarray(encodings_for_k[b].T, dtype=np.float32),
                "ev": np.ascontiguousarray(encodings_for_v[b].T, dtype=np.float32),
                "wq": wq,
                "wk": wk,
                "wv": wv,
                "mi": np.ascontiguousarray(mi),
            }
        )
        metas.append((b, q0s))

    res = run_bass_kernel_spmd(nc, in_maps, core_ids=list(range(8)))
    _CACHE["last_res"] = res

    out = np.empty((B, S, DM), dtype=np.float32)
    for c in range(8):
        b, q0s = metas[c]
        oT = res.results[c]["outT"]
        r = res.results[c]["rs"]
        for j, q0 in enumerate(q0s):
            blk = oT[:, j * 512 : (j + 1) * 512] / r[0, j * 512 : (j + 1) * 512][None, :]
            out[b, q0 : q0 + 512, :] = blk.T
    return out



# revision 5
# speedup vs baseline: 1.3723x; 1.3723x over previous
"""Causal attention head on 8 trn2 NeuronCores.

Sharding: core c = (batch b = c//2, type t = c%2). Each core handles 4
query stripes of 512 of its batch. Causal balance: type A gets stripes
[7,5,2,0] with real key-block counts R_A=[32,24,12,4]; type B stripes
[6,4,3,1] with R_B=[28,20,16,8]. One SPMD program: every core runs the
padded template T=[32,24,16,8]; per-core behaviour comes only from input
data (per-core threshold scalars select ones/triangle/zero masks).

Everything on the PE array is bf16 (no fp32 HIGH passes -> FWL stays
enabled, moving operands stream 1 row/cycle). Score matmuls are packed
two key-blocks per issue via PE row tiling (kT pairs live on partition
halves 0:64 / 64:128, qT is duplicated across both halves).

Per pair d (= template positions 2d, 2d+1) of slot j:
  ps[:,   0: 512] = kTp[ 0: 64, d].T @ qT[ 0: 64, slot]   (rows 0-63)
  ps[:, 512:1024] = kTp[64:128, d].T @ qT[64:128, slot]   (rows 64-127)
  e = exp(0.125*ps)                  bf16  (scalar engine)
  last 4 pairs: e = (QK >= thresh[j,m]) * e   (one fused DVE op; QK is
     an iota tile qi-kp-128*i2, thresh in {-1e9, 0, 256, 1e9} selects
     all-ones / causal triangles / all-zero per core+slot)
  acc[:, slot] += e                  fp16  (DVE/Pool)
  po0 += v[u][:,0:128].T @ e_half;  po1 += v[u][:,128:256].T @ e_half
Outputs: outT bf16 [256,2048] (unnormalized), acc fp16 [128,4096].
Host: r = colsum(acc) folded over pair halves; out = (outT/r).T.
"""

import sys

sys.path.insert(0, "/opt/trn_rl_repo")

import numpy as np
import ml_dtypes

B, S, DM, DQ = 4, 4096, 256, 64
T = [32, 24, 16, 8]  # padded template: key-blocks per slot
STRIPES_A = [7, 5, 2, 0]  # R_A = [32, 24, 12, 4]
STRIPES_B = [6, 4, 3, 1]  # R_B = [28, 20, 16, 8]
# Per-pair mask thresholds for the last 4 pairs of a slot.
# exact slot (R == T): pairs are [ones, ones, tri(0/128), tri(256/384)]
# padded slot (R == T-4): pairs are [tri(0/128), tri(256/384), zero, zero]
TH_EXACT = [-1e9, -1e9, 0.0, 256.0]
TH_PAD = [0.0, 256.0, 1e9, 1e9]

_CACHE = {}


def _build_nc():
    import concourse.bass as bass  # noqa: F401
    import concourse.tile as tile
    from concourse import bacc, mybir

    dt = mybir.dt
    f32, bf, f16 = dt.float32, dt.bfloat16, dt.float16

    nc = bacc.Bacc(
        "TRN2",
        target_bir_lowering=False,
        debug=False,
        enable_asserts=False,
        num_devices=8,
    )

    def din(name, shape, d):
        return nc.dram_tensor(name, shape, d, kind="ExternalInput").ap()

    eq = din("eq", [256, 2048], bf)
    ek = din("ek", [256, 4096], bf)  # column-reordered: even blocks, then odd
    ev = din("ev", [256, 4096], bf)
    wq = din("wq", [256, 128], bf)  # Wq.T duplicated along cols
    wk = din("wk", [256, 64], bf)  # Wk.T
    wv = din("wv", [256, 256], bf)  # Wv.T
    th = din("th", [128, 16], f32)  # mask thresholds per (slot, pair)
    outT = nc.dram_tensor("outT", [256, 2048], bf, kind="ExternalOutput").ap()
    acc_out = nc.dram_tensor("acc", [128, 4096], f16, kind="ExternalOutput").ap()

    Exp = mybir.ActivationFunctionType.Exp
    GE = mybir.AluOpType.is_ge
    MUL = mybir.AluOpType.mult

    with tile.TileContext(nc) as tc:
        from contextlib import ExitStack

        with ExitStack() as ctx:
            const = ctx.enter_context(tc.tile_pool(name="const", bufs=1))

            # ---- persistent SBUF tensors ----
            eq_sb = [const.tile([128, 2048], bf, tag=f"eq{h}", name=f"eq{h}") for h in range(2)]
            ek_sb = [const.tile([128, 4096], bf, tag=f"ek{h}", name=f"ek{h}") for h in range(2)]
            ev_sb = [const.tile([128, 4096], bf, tag=f"ev{h}", name=f"ev{h}") for h in range(2)]
            wq_sb = const.tile([128, 256], bf, tag="wq", name="wq")
            wk_sb = const.tile([128, 128], bf, tag="wk", name="wk")
            wv_sb = const.tile([128, 512], bf, tag="wv", name="wv")
            th_sb = const.tile([128, 16], f32, tag="th", name="th")
            qT = const.tile([128, 2048], bf, tag="qT", name="qT")  # dup halves
            kTp = const.tile([128, 2048], bf, tag="kTp", name="kTp")  # pair-packed
            v_sb = const.tile([128, 32 * 256], bf, tag="v", name="v")
            qk = const.tile([128, 1024], f16, tag="qk", name="qk")
            acc = const.tile([128, 4096], f16, tag="acc", name="acc")

            for h in range(2):
                nc.sync.dma_start(wq_sb[:, h * 128 : (h + 1) * 128], wq[h * 128 : (h + 1) * 128, :])
                nc.sync.dma_start(wk_sb[:, h * 64 : (h + 1) * 64], wk[h * 128 : (h + 1) * 128, :])
                nc.sync.dma_start(wv_sb[:, h * 256 : (h + 1) * 256], wv[h * 128 : (h + 1) * 128, :])
            nc.sync.dma_start(th_sb[:], th[:])
            for h in range(2):
                for c in range(4):
                    cs = slice(c * 512, (c + 1) * 512)
                    nc.sync.dma_start(eq_sb[h][:, cs], eq[h * 128 : (h + 1) * 128, cs])
            for h in range(2):
                for c in range(8):
                    cs = slice(c * 512, (c + 1) * 512)
                    nc.sync.dma_start(ek_sb[h][:, cs], ek[h * 128 : (h + 1) * 128, cs])
            for h in range(2):
                for c in range(8):
                    cs = slice(c * 512, (c + 1) * 512)
                    nc.sync.dma_start(ev_sb[h][:, cs], ev[h * 128 : (h + 1) * 128, cs])

            # QK[p, i2*512 + qi] = qi - 128*i2 - p
            nc.gpsimd.iota(
                qk[:],
                [[-128, 2], [1, 512]],
                base=0,
                channel_multiplier=-1,
                allow_small_or_imprecise_dtypes=True,
            )

            # ---- projections ----
            with tc.tile_pool(name="pp", bufs=3, space="PSUM") as pp:
                # qT (duplicated onto both partition halves by the dup'd wq)
                for c in range(4):
                    ps = pp.tile([128, 512], f32, tag="ps", name="ps")
                    for h in range(2):
                        nc.tensor.matmul(
                            ps[:],
                            wq_sb[:, h * 128 : (h + 1) * 128],
                            eq_sb[h][:, c * 512 : (c + 1) * 512],
                            start=(h == 0),
                            stop=(h == 1),
                        )
                    nc.vector.tensor_copy(qT[:, c * 512 : (c + 1) * 512], ps[:])
                # kTp: even blocks -> partitions 0:64, odd -> 64:128
                for c in range(4):
                    ps = pp.tile([128, 512], f32, tag="ps", name="ps")
                    for half in range(2):  # 0: even->top, 1: odd->bottom
                        dst = ps[half * 64 : (half + 1) * 64, :]
                        for h in range(2):
                            nc.tensor.matmul(
                                dst,
                                wk_sb[:, h * 64 : (h + 1) * 64],
                                ek_sb[h][:, half * 2048 + c * 512 : half * 2048 + (c + 1) * 512],
                                start=(h == 0),
                                stop=(h == 1),
                            )
                    nc.vector.tensor_copy(kTp[:, c * 512 : (c + 1) * 512], ps[:])
                # v natural [keys, 256] in bf16, two blocks per psum tile
                for i in range(16):
                    ps = pp.tile([128, 512], f32, tag="ps", name="ps")
                    for s in range(2):
                        t = 2 * i + s
                        for h in range(2):
                            nc.tensor.matmul(
                                ps[:, s * 256 : (s + 1) * 256],
                                ev_sb[h][:, t * 128 : (t + 1) * 128],
                                wv_sb[:, h * 256 : (h + 1) * 256],
                                start=(h == 0),
                                stop=(h == 1),
                            )
                    nc.scalar.copy(v_sb[:, i * 512 : (i + 1) * 512], ps[:])

            # ---- attention ----
            psc = ctx.enter_context(tc.tile_pool(name="psc", bufs=2, space="PSUM"))
            po_pool = ctx.enter_context(tc.tile_pool(name="po", bufs=2, space="PSUM"))
            epool = ctx.enter_context(tc.tile_pool(name="e", bufs=4))
            opool = ctx.enter_context(tc.tile_pool(name="o", bufs=2))

            LAG = 2
            acc_n = 0
            for j in (3, 2, 1, 0):
                npairs = T[j] // 2
                po0 = po_pool.tile([128, 512], f32, tag="po0", name="po0")
                po1 = po_pool.tile([128, 512], f32, tag="po1", name="po1")
                qs_top = qT[0:64, j * 512 : (j + 1) * 512]
                qs_bot = qT[64:128, j * 512 : (j + 1) * 512]
                acc_j = acc[:, j * 1024 : (j + 1) * 1024]
                es = [None] * npairs
                for d in range(npairs + LAG):
                    if d < npairs:
                        ps = psc.tile([128, 1024], f32, tag="ps", name="ps")
                        nc.tensor.matmul(
                            ps[:, 0:512],
                            kTp[0:64, d * 128 : (d + 1) * 128],
                            qs_top,
                            start=True,
                            stop=True,
                        )
                        nc.tensor.matmul(
                            ps[:, 512:1024],
                            kTp[64:128, d * 128 : (d + 1) * 128],
                            qs_bot,
                            start=True,
                            stop=True,
                        )
                        e = epool.tile([128, 1024], bf, tag="e", name="e")
                        nc.scalar.activation(e[:], ps[:], Exp, scale=0.125)
                        if d >= npairs - 4:
                            m = d - (npairs - 4)
                            col = j * 4 + m
                            nc.vector.scalar_tensor_tensor(
                                e[:], qk[:], th_sb[:, col : col + 1], e[:], GE, MUL
                            )
                        if d == 0:
                            nc.vector.tensor_copy(acc_j, e[:])
                        else:
                            eng = nc.gpsimd if acc_n % 7 == 6 else nc.vector
                            eng.tensor_add(acc_j, acc_j, e[:])
                        acc_n += 1
                        es[d] = e
                    if d >= LAG:
                        dd = d - LAG
                        e = es[dd]
                        for half in range(2):
                            u = 2 * dd + half
                            eh = e[:, half * 512 : (half + 1) * 512]
                            first = dd == 0 and half == 0
                            last = dd == npairs - 1 and half == 1
                            nc.tensor.matmul(
                                po0[:],
                                v_sb[:, u * 256 : u * 256 + 128],
                                eh,
                                start=first,
                                stop=last,
                            )
                            nc.tensor.matmul(
                                po1[:],
                                v_sb[:, u * 256 + 128 : (u + 1) * 256],
                                eh,
                                start=first,
                                stop=last,
                            )
                o0 = opool.tile([128, 512], bf, tag="o0", name="o0")
                o1 = opool.tile([128, 512], bf, tag="o1", name="o1")
                nc.vector.tensor_copy(o0[:], po0[:])
                nc.vector.tensor_copy(o1[:], po1[:])
                nc.sync.dma_start(outT[0:128, j * 512 : (j + 1) * 512], o0[:])
                nc.sync.dma_start(outT[128:256, j * 512 : (j + 1) * 512], o1[:])
                nc.sync.dma_start(acc_out[:, j * 1024 : (j + 1) * 1024], acc_j)

    nc.compile()
    return nc


def kernel(encodings_for_q, encodings_for_k, encodings_for_v, mask, Wq, Wk, Wv):
    from concourse.bass_utils import run_bass_kernel_spmd

    if "nc" not in _CACHE:
        _CACHE["nc"] = _build_nc()
    nc = _CACHE["nc"]

    bf = ml_dtypes.bfloat16
    wqd = np.ascontiguousarray(
        np.concatenate([Wq.T, Wq.T], axis=1), dtype=bf
    )  # [256,128]
    wkt = np.ascontiguousarray(Wk.T, dtype=bf)  # [256,64]
    wvt = np.ascontiguousarray(Wv.T, dtype=bf)  # [256,256]

    in_maps = []
    metas = []
    for c in range(8):
        b, t = c // 2, c % 2
        stripes = STRIPES_A if t == 0 else STRIPES_B
        eqT = np.concatenate(
            [encodings_for_q[b, st * 512 : (st + 1) * 512, :].T for st in stripes],
            axis=1,
        )
        ekT = encodings_for_k[b].T.reshape(256, 32, 128)
        ek_reord = np.concatenate([ekT[:, 0::2, :], ekT[:, 1::2, :]], axis=1).reshape(
            256, 4096
        )
        # thresholds: slot j exact if R[j] == T[j]
        thr = np.empty((16,), dtype=np.float32)
        for j in range(4):
            R = 4 * (stripes[j] + 1)
            vals = TH_EXACT if R == T[j] else TH_PAD
            thr[j * 4 : (j + 1) * 4] = vals
        in_maps.append(
            {
                "eq": np.ascontiguousarray(eqT, dtype=bf),
                "ek": np.ascontiguousarray(ek_reord, dtype=bf),
                "ev": np.ascontiguousarray(encodings_for_v[b].T, dtype=bf),
                "wq": wqd,
                "wk": wkt,
                "wv": wvt,
                "th": np.ascontiguousarray(np.broadcast_to(thr, (128, 16))),
            }
        )
        metas.append((b, stripes))

    res = run_bass_kernel_spmd(nc, in_maps, core_ids=list(range(8)))
    _CACHE["last_res"] = res

    out = np.empty((B, S, DM), dtype=np.float32)
    for c in range(8):
        b, stripes = metas[c]
        oT = res.results[c]["outT"].astype(np.float32)
        a = res.results[c]["acc"].astype(np.float32)
        for j, st in enumerate(stripes):
            r = a[:, j * 1024 : j * 1024 + 512].sum(0) + a[
                :, j * 1024 + 512 : (j + 1) * 1024
            ].sum(0)
            blk = oT[:, j * 512 : (j + 1) * 512] / r[None, :]
            out[b, st * 512 : (st + 1) * 512, :] = blk.T
    return out
